# revision 5
# baseline (speedup 1.0000x reference)
"""ALiBi attention (B=2, S=2048, H=16, Dh=64) on 8 TRN2 NeuronCores.

Sharding: head-parallel attention (2 heads x 2 batches per core), qkv
column-sharded, out-projection K-sharded (per-core partial summed on host).
No collectives. All heavy matmuls in bf16 with f32 PSUM accumulation.

One SPMD graph shared by all 8 cores: everything slope/head-dependent
(ALiBi bias tiles, per-tile exp offsets) arrives via per-core inputs.

Layouts (device, per core c; heads h0=2c, h1=2c+1):
  xT_sb   [128, 8*4096]  bf16  x^T, hid-chunk-major (host pre-transposed)
  qT/kT   [64, 2*4096]   bf16  per-head transposed q/k: free = hl*4096 + b*2048 + s
  v_sb    [128, 32*130]  bf16  v rows chunked; per chunk [v_h0*mask |mask| v_h1*mask |mask]
  scores  [128k, 512q]   f32   PSUM, transposed layout => the softmax denominator
                               falls out of the PV matmul via the mask column of V
  attn_T  [128, 4096]    bf16  normalized attention, transposed = lhsT of out-proj
"""

import math
import numpy as np
import ml_dtypes

bf16 = ml_dtypes.bfloat16

HID, H, DH = 1024, 16, 64
B, S = 2, 2048
NCORES = 8
NLIN = 12  # linear deltas: 128..1536 step 128 -> index j = delta//128 - 1


def _alibi_slopes(n_head):
    main = 2 ** int(math.log2(n_head))
    m = (2.0 ** (-8.0 / main)) ** np.arange(1, 1 + main)
    if main < n_head:
        intra = (2.0 ** (-4.0 / main)) ** np.arange(1, 1 + 2 * (n_head - main), 2)
        m = np.concatenate([m, intra])
    return m.astype(np.float32)


def _classify(delta):
    """Bias class of a [128k x 512q] score tile at q0-k0=delta."""
    if delta <= -512:
        return ("zero", None)
    if delta >= 128:
        return ("linear", delta // 128 - 1)
    return ("mixed", (-delta) // 128)  # delta in {0,-128,-256,-384} -> idx 0..3


def build_nc():
    import concourse.tile as tile
    from concourse import bacc, mybir

    f32 = mybir.dt.float32
    bf = mybir.dt.bfloat16
    AF = mybir.ActivationFunctionType
    ALU = mybir.AluOpType

    nc = bacc.Bacc("TRN2", target_bir_lowering=False, debug=False,
                   enable_asserts=False, num_devices=NCORES)

    # ---- DRAM I/O ----
    xT_d = nc.dram_tensor("xT", [8, 128, 4096], bf, kind="ExternalInput").ap()
    wq_d = nc.dram_tensor("wq", [8, 128, 128], bf, kind="ExternalInput").ap()
    wk_d = nc.dram_tensor("wk", [8, 128, 128], bf, kind="ExternalInput").ap()
    wv_d = nc.dram_tensor("wv", [8, 128, 128], bf, kind="ExternalInput").ap()
    wout_d = nc.dram_tensor("wout", [128, 1024], bf, kind="ExternalInput").ap()
    # btiles: per head-slot hl: 4 mixed relu tiles + 1 linear tile, each [128,512]
    bt_d = nc.dram_tensor("btiles", [128, 10 * 512], f32, kind="ExternalInput").ap()
    # lconst[p, hl*NLIN + j] = -slope_hl * 128*(j+1), replicated across partitions
    lc_d = nc.dram_tensor("lconst", [128, 2 * NLIN], f32, kind="ExternalInput").ap()
    mk_d = nc.dram_tensor("maskf", [128, 32], f32, kind="ExternalInput").ap()

    out_d = nc.dram_tensor("out_p", [2, 16, 128, 1024], bf, kind="ExternalOutput").ap()
    k_d = nc.dram_tensor("k_out", [64, 8192], bf, kind="ExternalOutput").ap()
    v_d = nc.dram_tensor("v_out", [128, 32, 128], bf, kind="ExternalOutput").ap()

    with tile.TileContext(nc) as tc:
        import contextlib
        with contextlib.ExitStack() as ctx:
            persist = ctx.enter_context(tc.tile_pool(name="persist", bufs=1))
            qT = persist.tile([64, 8192], bf, tag="qT")
            kT = persist.tile([64, 8192], bf, tag="kT")
            v_sb = persist.tile([128, 32 * 130], bf, tag="v_sb")
            attn_T = persist.tile([128, 4096], bf, tag="attn_T")
            wout_sb = persist.tile([128, 1024], bf, tag="wout")
            btiles = persist.tile([128, 10 * 512], f32, tag="btiles")
            lconst = persist.tile([128, 2 * NLIN], f32, tag="lconst")
            maskf = persist.tile([128, 32], f32, tag="maskf")
            ones_sb = persist.tile([1, 64], f32, tag="ones")

            nc.sync.dma_start(wout_sb[:], wout_d[:])
            nc.sync.dma_start(btiles[:], bt_d[:])
            nc.sync.dma_start(lconst[:], lc_d[:])
            nc.sync.dma_start(maskf[:], mk_d[:])
            nc.vector.memset(ones_sb[:], 1.0)

            # mask columns of v_sb (positions 64 and 129 of each 130-chunk)
            v3 = v_sb.rearrange("p (c w) -> p c w", w=130)
            mk3 = maskf.rearrange("p (c o) -> p c o", o=1)
            nc.vector.tensor_copy(v3[:, :, 64:65], mk3[:])
            nc.vector.tensor_copy(v3[:, :, 129:130], mk3[:])

            # ================= Phase 1: QKV projection =================
            with contextlib.ExitStack() as ph1:
                xw = ph1.enter_context(tc.tile_pool(name="xw", bufs=1))
                vop = ph1.enter_context(tc.tile_pool(name="vop", bufs=3))
                ps_qkv = ph1.enter_context(tc.tile_pool(name="ps_qkv", bufs=2, space="PSUM"))
                xT_sb = xw.tile([128, 8 * 4096], bf, tag="xT")
                wq_sb = xw.tile([128, 8 * 128], bf, tag="wq")
                wk_sb = xw.tile([128, 8 * 128], bf, tag="wk")
                wv_sb = xw.tile([128, 8 * 128], bf, tag="wv")
                for h in range(8):
                    nc.sync.dma_start(xT_sb[:, h * 4096:(h + 1) * 4096], xT_d[h])
                    nc.sync.dma_start(wq_sb[:, h * 128:(h + 1) * 128], wq_d[h])
                    nc.sync.dma_start(wk_sb[:, h * 128:(h + 1) * 128], wk_d[h])
                    nc.sync.dma_start(wv_sb[:, h * 128:(h + 1) * 128], wv_d[h])

                # q, k transposed (form 2: W stationary, xT moving)
                for w_sb, dst in ((wq_sb, qT), (wk_sb, kT)):
                    for n in range(8):  # 512-row chunks of (b,s)
                        ps = ps_qkv.tile([128, 512], f32, tag="qk")
                        for h in range(8):
                            nc.tensor.matmul(
                                ps[:],
                                lhsT=w_sb[:, h * 128:(h + 1) * 128],
                                rhs=xT_sb[:, h * 4096 + n * 512: h * 4096 + (n + 1) * 512],
                                start=(h == 0), stop=(h == 7))
                        for hl in range(2):
                            nc.vector.tensor_copy(
                                dst[:, hl * 4096 + n * 512: hl * 4096 + (n + 1) * 512],
                                ps[hl * 64:(hl + 1) * 64, :])

                # v natural (form 1: xT stationary, Wv moving) + masked copy
                for r in range(32):  # 128-row chunks
                    ps = ps_qkv.tile([128, 128], f32, tag="v")
                    for h in range(8):
                        nc.tensor.matmul(
                            ps[:],
                            lhsT=xT_sb[:, h * 4096 + r * 128: h * 4096 + r * 128 + 128],
                            rhs=wv_sb[:, h * 128:(h + 1) * 128],
                            start=(h == 0), stop=(h == 7))
                    for hl in range(2):
                        nc.vector.tensor_scalar_mul(
                            v3[:, r, hl * 65: hl * 65 + 64],
                            ps[:, hl * 64:(hl + 1) * 64],
                            maskf[:, r:r + 1])
                    vo = vop.tile([128, 128], bf, tag="vo")
                    nc.any.tensor_copy(vo[:], ps[:])
                    nc.sync.dma_start(v_d[:, r, :], vo[:])

                nc.sync.dma_start(k_d[:], kT[:])

            # ============ Phase 2+3: attention + out-projection ============
            with contextlib.ExitStack() as ph2:
                sc_pool = ph2.enter_context(tc.tile_pool(name="sc", bufs=2, space="PSUM"))
                ot_pool = ph2.enter_context(tc.tile_pool(name="ot", bufs=2, space="PSUM"))
                bc_pool = ph2.enter_context(tc.tile_pool(name="bc", bufs=1, space="PSUM"))
                op_pool = ph2.enter_context(tc.tile_pool(name="op", bufs=1, space="PSUM"))
                work = ph2.enter_context(tc.tile_pool(name="work", bufs=3))
                work2 = ph2.enter_context(tc.tile_pool(name="work2", bufs=2))
                for b in range(B):
                    for qc in range(4):
                        q0 = b * 2048 + qc * 512
                        otiles = [ot_pool.tile([65, 512], f32, tag="ot", name=f"ot{b}_{qc}_{hl}")
                                  for hl in range(2)]
                        for kc in range(16):
                            delta = qc * 512 - kc * 128
                            cls, cidx = _classify(delta)
                            sw = sc_pool.tile([128, 1024], f32, tag="sc")
                            for hl in range(2):
                                koff = hl * 4096 + b * 2048 + kc * 128
                                qoff = hl * 4096 + b * 2048 + qc * 512
                                nc.tensor.matmul(
                                    sw[:, hl * 512:(hl + 1) * 512],
                                    lhsT=kT[:, koff:koff + 128],
                                    rhs=qT[:, qoff:qoff + 512],
                                    start=True, stop=True)
                            pw = work.tile([128, 1024], bf, tag="probs")
                            if cls == "zero":
                                src = sw
                            else:
                                tmp = work2.tile([128, 1024], f32, tag="tmp")
                                for hl in range(2):
                                    if cls == "linear":
                                        bslice = btiles[:, (hl * 5 + 4) * 512:(hl * 5 + 5) * 512]
                                        sconst = lconst[:, hl * NLIN + cidx: hl * NLIN + cidx + 1]
                                    else:
                                        bslice = btiles[:, (hl * 5 + cidx) * 512:(hl * 5 + cidx + 1) * 512]
                                        sconst = 0.0
                                    nc.vector.scalar_tensor_tensor(
                                        tmp[:, hl * 512:(hl + 1) * 512],
                                        sw[:, hl * 512:(hl + 1) * 512],
                                        sconst, bslice, ALU.add, ALU.add)
                                src = tmp
                            nc.scalar.activation(pw[:], src[:], AF.Exp)
                            ch = b * 16 + kc
                            for hl in range(2):
                                nc.tensor.matmul(
                                    otiles[hl][:],
                                    lhsT=v_sb[:, ch * 130 + hl * 65: ch * 130 + hl * 65 + 65],
                                    rhs=pw[:, hl * 512:(hl + 1) * 512],
                                    start=(kc == 0), stop=(kc == 15))
                        # normalize: attn = num / den
                        for hl in range(2):
                            rc = work.tile([1, 512], f32, tag="recip")
                            nc.vector.reciprocal(rc[:], otiles[hl][64:65, :])
                            bcp = bc_pool.tile([64, 512], f32, tag="bc")
                            nc.tensor.matmul(bcp[:], lhsT=ones_sb[:], rhs=rc[:],
                                             start=True, stop=True)
                            at = work.tile([64, 512], f32, tag="atmp")
                            nc.scalar.copy(at[:], otiles[hl][0:64, :])
                            nc.vector.tensor_mul(
                                attn_T[hl * 64:(hl + 1) * 64, q0:q0 + 512],
                                at[:], bcp[:])

                    # out-projection for this batch (overlaps next batch's attention)
                    for r in range(16):
                        for n2 in range(2):
                            ps = op_pool.tile([128, 512], f32, tag="op")
                            nc.tensor.matmul(
                                ps[:],
                                lhsT=attn_T[:, b * 2048 + r * 128: b * 2048 + r * 128 + 128],
                                rhs=wout_sb[:, n2 * 512:(n2 + 1) * 512],
                                start=True, stop=True)
                            st = work.tile([128, 512], bf, tag="ostage")
                            nc.any.tensor_copy(st[:], ps[:])
                            nc.sync.dma_start(out_d[b, r, :, n2 * 512:(n2 + 1) * 512], st[:])

    nc.compile()
    return nc


_COMPILED = None


def _get_nc():
    global _COMPILED
    if _COMPILED is None:
        _COMPILED = build_nc()
    return _COMPILED


def _host_inputs(x, mask, Wqkv, Wout):
    slopes = _alibi_slopes(H)
    xf = np.ascontiguousarray(np.asarray(x, np.float32).reshape(B * S, HID))
    xT = np.ascontiguousarray(xf.T).astype(bf16).reshape(8, 128, 4096)
    maskf = np.asarray(mask, np.float32).reshape(B * S)
    mk = np.ascontiguousarray(maskf.reshape(32, 128).T)  # [128p, 32c]

    kk = np.arange(128, dtype=np.float32)[:, None]
    qq = np.arange(512, dtype=np.float32)[None, :]
    base = qq - kk  # [128, 512]

    Wqkv = np.asarray(Wqkv, np.float32)
    Wout = np.asarray(Wout, np.float32)

    in_maps = []
    for c in range(NCORES):
        h0 = 2 * c
        cols = slice(h0 * 64, h0 * 64 + 128)
        wq = np.ascontiguousarray(Wqkv[:, cols]).astype(bf16).reshape(8, 128, 128)
        wk = np.ascontiguousarray(Wqkv[:, 1024 + h0 * 64: 1024 + h0 * 64 + 128]
                                  ).astype(bf16).reshape(8, 128, 128)
        wv = np.ascontiguousarray(Wqkv[:, 2048 + h0 * 64: 2048 + h0 * 64 + 128]
                                  ).astype(bf16).reshape(8, 128, 128)
        wo = np.ascontiguousarray(Wout[c * 128:(c + 1) * 128, :]).astype(bf16)
        bt = np.empty((128, 10 * 512), np.float32)
        lc = np.empty((128, 2 * NLIN), np.float32)
        for hl in range(2):
            sl = float(slopes[h0 + hl])
            for ci, delta in enumerate((0, -128, -256, -384)):
                bt[:, (hl * 5 + ci) * 512:(hl * 5 + ci + 1) * 512] = \
                    -sl * np.maximum(base + delta, 0.0)
            bt[:, (hl * 5 + 4) * 512:(hl * 5 + 5) * 512] = -sl * base
            for j in range(NLIN):
                lc[:, hl * NLIN + j] = -sl * 128.0 * (j + 1)
        in_maps.append({
            "xT": xT, "wq": wq, "wk": wk, "wv": wv, "wout": wo,
            "btiles": bt, "lconst": lc, "maskf": mk,
        })
    return in_maps


def kernel(x, mask, Wqkv, Wout, trace=False):
    from concourse.bass_utils import run_bass_kernel_spmd

    nc = _get_nc()
    in_maps = _host_inputs(x, mask, Wqkv, Wout)
    res = run_bass_kernel_spmd(nc, in_maps, core_ids=list(range(NCORES)), trace=trace)
    results = res.results

    out = np.zeros((B * S, HID), np.float32)
    k_cache = np.empty((B * S, HID), np.float32)
    v_cache = np.empty((B * S, HID), np.float32)
    for c in range(NCORES):
        r = results[c]
        out += r["out_p"].astype(np.float32).reshape(B * S, HID)
        ko = r["k_out"].astype(np.float32)  # [64, 8192]
        for hl in range(2):
            k_cache[:, c * 128 + hl * 64: c * 128 + (hl + 1) * 64] = \
                ko[:, hl * 4096:(hl + 1) * 4096].T
        v_cache[:, c * 128:(c + 1) * 128] = \
            r["v_out"].astype(np.float32).transpose(1, 0, 2).reshape(B * S, 128)
    out = out.reshape(B, S, HID)
    k_cache = k_cache.reshape(B, S, HID)
    v_cache = v_cache.reshape(B, S, HID)
    if trace:
        return (out, k_cache, v_cache), res
    return out, k_cache, v_cache


# revision 15
# speedup vs baseline: 1.3597x; 1.3597x over previous
"""ALiBi attention (B=2, S=2048, H=16, Dh=64) on 8 TRN2 NeuronCores.

Sharding: head-parallel attention (2 heads x 2 batches per core), qkv
column-sharded, out-projection K-sharded (per-core partial summed on host).
No collectives. All heavy matmuls in bf16 with f32 PSUM accumulation.

One SPMD graph shared by all 8 cores: everything slope/head-dependent
(exp(ALiBi-bias) tiles) arrives via per-core inputs.

Softmax is computed in transposed score layout [k,q]; the denominator
falls out of the PV matmul via a mask column appended to V. The ALiBi
bias is applied multiplicatively AFTER exp (probs = exp(s)*exp(bias))
on the otherwise-idle GPSIMD engine; exp(bias) tiles are host-precomputed
per (head, tile-diagonal-offset) — multiplicative bf16 error is ~0.4%
with no cancellation hazard.
"""

import math
import numpy as np
import ml_dtypes

bf16 = ml_dtypes.bfloat16

HID, H, DH = 1024, 16, 64
B, S = 2, 2048
NCORES = 8
NEB = 16  # exp-bias tile-pair classes: 4 mixed (delta 0..-384) + 12 linear (128..1536)


def _alibi_slopes(n_head):
    main = 2 ** int(math.log2(n_head))
    m = (2.0 ** (-8.0 / main)) ** np.arange(1, 1 + main)
    if main < n_head:
        intra = (2.0 ** (-4.0 / main)) ** np.arange(1, 1 + 2 * (n_head - main), 2)
        m = np.concatenate([m, intra])
    return m.astype(np.float32)


def _eb_idx(delta):
    """exp-bias class index for tile diagonal offset delta, or None if bias==0."""
    if delta <= -512:
        return None
    if delta >= 128:
        return delta // 128 + 3  # 4..15
    return (-delta) // 128  # 0..3


def build_nc():
    import concourse.tile as tile
    from concourse import bacc, mybir

    f32 = mybir.dt.float32
    bf = mybir.dt.bfloat16
    AF = mybir.ActivationFunctionType

    nc = bacc.Bacc("TRN2", target_bir_lowering=False, debug=False,
                   enable_asserts=False, num_devices=NCORES)

    # ---- DRAM I/O ----
    xT_d = nc.dram_tensor("xT", [8, 128, 4096], bf, kind="ExternalInput").ap()
    wq_d = nc.dram_tensor("wq", [8, 128, 128], bf, kind="ExternalInput").ap()
    wk_d = nc.dram_tensor("wk", [8, 128, 128], bf, kind="ExternalInput").ap()
    wv_d = nc.dram_tensor("wv", [8, 128, 128], bf, kind="ExternalInput").ap()
    wout_d = nc.dram_tensor("wout", [128, 1024], bf, kind="ExternalInput").ap()
    eb_d = nc.dram_tensor("ebias", [128, NEB * 1024], bf, kind="ExternalInput").ap()
    mk_d = nc.dram_tensor("maskf", [128, 32], f32, kind="ExternalInput").ap()

    out_d = nc.dram_tensor("out_p", [2, 16, 128, 1024], bf, kind="ExternalOutput").ap()
    k_d = nc.dram_tensor("k_out", [64, 8192], bf, kind="ExternalOutput").ap()
    v_d = nc.dram_tensor("v_out", [128, 32, 128], bf, kind="ExternalOutput").ap()

    with tile.TileContext(nc) as tc:
        import contextlib
        with contextlib.ExitStack() as ctx:
            persist = ctx.enter_context(tc.tile_pool(name="persist", bufs=1))
            qT = persist.tile([64, 8192], bf, tag="qT")
            kT = persist.tile([64, 8192], bf, tag="kT")
            v_sb = persist.tile([128, 32 * 130], bf, tag="v_sb")
            attn_T = persist.tile([128, 4096], bf, tag="attn_T")
            wout_sb = persist.tile([128, 1024], bf, tag="wout")
            eb_sb = persist.tile([128, NEB * 1024], bf, tag="eb")
            maskf = persist.tile([128, 32], f32, tag="maskf")
            atst = persist.tile([64, 8 * 512], f32, tag="atst")   # numerators (1 batch)
            # denominators: slot (qc,hl) -> partition 32*qc, col hl*512
            # (compute writes must start at 32-aligned partitions)
            den = persist.tile([128, 1024], f32, tag="den")
            rcp = persist.tile([128, 1024], f32, tag="rcp")
            ones_sb = persist.tile([1, 64], f32, tag="ones")

            xw = ctx.enter_context(tc.tile_pool(name="xw", bufs=1))
            xT_sb = xw.tile([128, 8 * 4096], bf, tag="xT")
            wq_sb = xw.tile([128, 8 * 128], bf, tag="wq")
            wk_sb = xw.tile([128, 8 * 128], bf, tag="wk")
            wv_sb = xw.tile([128, 8 * 128], bf, tag="wv")

            work = ctx.enter_context(tc.tile_pool(name="work", bufs=3))
            work2 = ctx.enter_context(tc.tile_pool(name="work2", bufs=2))
            ps_sc = ctx.enter_context(tc.tile_pool(name="ps_sc", bufs=2, space="PSUM"))
            ps_ot = ctx.enter_context(tc.tile_pool(name="ps_ot", bufs=2, space="PSUM"))
            ps_ms = ctx.enter_context(tc.tile_pool(name="ps_ms", bufs=2, space="PSUM"))

            # ---- input DMAs (weights first, then x for batch 0, then rest) ----
            for h in range(8):
                nc.sync.dma_start(wq_sb[:, h * 128:(h + 1) * 128], wq_d[h])
                nc.sync.dma_start(wk_sb[:, h * 128:(h + 1) * 128], wk_d[h])
                nc.sync.dma_start(wv_sb[:, h * 128:(h + 1) * 128], wv_d[h])
            for h in range(8):
                nc.sync.dma_start(xT_sb[:, h * 4096: h * 4096 + 2048], xT_d[h, :, 0:2048])
            nc.sync.dma_start(wout_sb[:], wout_d[:])
            nc.sync.dma_start(maskf[:], mk_d[:])
            for h in range(8):
                nc.sync.dma_start(xT_sb[:, h * 4096 + 2048: (h + 1) * 4096],
                                  xT_d[h, :, 2048:4096])
            for j in range(4):
                nc.sync.dma_start(eb_sb[:, j * 4096:(j + 1) * 4096],
                                  eb_d[:, j * 4096:(j + 1) * 4096])

            # mask columns of v_sb (positions 64 and 129 of each 130-chunk)
            v3 = v_sb.rearrange("p (c w) -> p c w", w=130)
            mk3 = maskf.rearrange("p (c o) -> p c o", o=1)
            nc.vector.tensor_copy(v3[:, :, 64:65], mk3[:])
            nc.vector.tensor_copy(v3[:, :, 129:130], mk3[:])
            nc.vector.memset(den[:], 1.0)  # unused slots stay finite for reciprocal
            nc.vector.memset(ones_sb[:], 1.0)

            def qkv_phase(b):
                for w_sb, dst in ((wq_sb, qT), (wk_sb, kT)):
                    for n in range(4 * b, 4 * b + 4):  # 512-row chunks
                        ps = ps_ms.tile([128, 512], f32, tag="ms", name=f"qk{b}_{n}")
                        for h in range(8):
                            nc.tensor.matmul(
                                ps[:],
                                lhsT=w_sb[:, h * 128:(h + 1) * 128],
                                rhs=xT_sb[:, h * 4096 + n * 512: h * 4096 + (n + 1) * 512],
                                start=(h == 0), stop=(h == 7))
                        for hl in range(2):
                            nc.vector.tensor_copy(
                                dst[:, hl * 4096 + n * 512: hl * 4096 + (n + 1) * 512],
                                ps[hl * 64:(hl + 1) * 64, :])
                vo = None
                for r in range(16 * b, 16 * b + 16):  # 128-row chunks
                    if r % 4 == 0:
                        vo = work2.tile([128, 512], bf, tag="vo", name=f"vo{r}")
                    ps = ps_ms.tile([128, 128], f32, tag="ms", name=f"v{b}_{r}")
                    for h in range(8):
                        nc.tensor.matmul(
                            ps[:],
                            lhsT=xT_sb[:, h * 4096 + r * 128: h * 4096 + r * 128 + 128],
                            rhs=wv_sb[:, h * 128:(h + 1) * 128],
                            start=(h == 0), stop=(h == 7))
                    for hl in range(2):
                        nc.vector.tensor_scalar_mul(
                            v3[:, r, hl * 65: hl * 65 + 64],
                            ps[:, hl * 64:(hl + 1) * 64],
                            maskf[:, r:r + 1])
                    nc.any.tensor_copy(vo[:, (r % 4) * 128:(r % 4) * 128 + 128], ps[:])
                    if r % 4 == 3:
                        nc.sync.dma_start(v_d[:, r - 3:r + 1, :],
                                          vo.rearrange("p (c w) -> p c w", w=128)[:])

            def attention_phase(b):
                for qc in range(4):
                    q0 = b * 2048 + qc * 512
                    otiles = [ps_ot.tile([65, 512], f32, tag="ot", name=f"ot{b}_{qc}_{hl}")
                              for hl in range(2)]
                    for kc in range(16):
                        delta = qc * 512 - kc * 128
                        ei = _eb_idx(delta)
                        sw = ps_sc.tile([128, 1024], f32, tag="sc", name=f"sw{b}_{qc}_{kc}")
                        for hl in range(2):
                            koff = hl * 4096 + b * 2048 + kc * 128
                            qoff = hl * 4096 + b * 2048 + qc * 512
                            nc.tensor.matmul(
                                sw[:, hl * 512:(hl + 1) * 512],
                                lhsT=kT[:, koff:koff + 128],
                                rhs=qT[:, qoff:qoff + 512],
                                start=True, stop=True)
                        pw = work.tile([128, 1024], bf, tag="pw", name=f"pw{b}_{qc}_{kc}")
                        nc.scalar.activation(pw[:], sw[:], AF.Exp)
                        if ei is not None:
                            pb = work.tile([128, 1024], bf, tag="pb", name=f"pb{b}_{qc}_{kc}")
                            nc.gpsimd.tensor_mul(pb[:], pw[:],
                                                 eb_sb[:, ei * 1024:(ei + 1) * 1024])
                            src = pb
                        else:
                            src = pw
                        ch = b * 16 + kc
                        for hl in range(2):
                            nc.tensor.matmul(
                                otiles[hl][:],
                                lhsT=v_sb[:, ch * 130 + hl * 65: ch * 130 + hl * 65 + 65],
                                rhs=src[:, hl * 512:(hl + 1) * 512],
                                start=(kc == 0), stop=(kc == 15))
                    for hl in range(2):
                        slot = qc * 2 + hl
                        nc.vector.tensor_copy(
                            den[32 * qc:32 * qc + 1, hl * 512:(hl + 1) * 512],
                            otiles[hl][64:65, :])
                        nc.vector.tensor_copy(atst[:, slot * 512:(slot + 1) * 512],
                                              otiles[hl][0:64, :])

            def normalize_and_outproj(b):
                nc.vector.reciprocal(rcp[:], den[:])
                for qc in range(4):
                    q0 = b * 2048 + qc * 512
                    for hl in range(2):
                        slot = qc * 2 + hl
                        rc0 = work.tile([1, 512], f32, tag="rc0", name=f"rc{b}_{slot}")
                        nc.vector.tensor_copy(
                            rc0[:], rcp[32 * qc:32 * qc + 1, hl * 512:(hl + 1) * 512])
                        bc = ps_ms.tile([64, 512], f32, tag="ms", name=f"bc{b}_{slot}")
                        nc.tensor.matmul(bc[:], lhsT=ones_sb[:], rhs=rc0[:],
                                         start=True, stop=True)
                        nc.vector.tensor_mul(
                            attn_T[hl * 64:(hl + 1) * 64, q0:q0 + 512],
                            atst[:, slot * 512:(slot + 1) * 512], bc[:])
                for r in range(16):
                    st = work.tile([128, 1024], bf, tag="ostage", name=f"st{b}_{r}")
                    for n2 in range(2):
                        ps = ps_ms.tile([128, 512], f32, tag="ms", name=f"op{b}_{r}_{n2}")
                        nc.tensor.matmul(
                            ps[:],
                            lhsT=attn_T[:, b * 2048 + r * 128: b * 2048 + r * 128 + 128],
                            rhs=wout_sb[:, n2 * 512:(n2 + 1) * 512],
                            start=True, stop=True)
                        nc.any.tensor_copy(st[:, n2 * 512:(n2 + 1) * 512], ps[:])
                    nc.sync.dma_start(out_d[b, r], st[:])

            qkv_phase(0)
            attention_phase(0)
            qkv_phase(1)
            nc.sync.dma_start(k_d[:], kT[:])
            normalize_and_outproj(0)
            attention_phase(1)
            normalize_and_outproj(1)

    nc.compile()
    return nc


_COMPILED = None


def _get_nc():
    global _COMPILED
    if _COMPILED is None:
        _COMPILED = build_nc()
    return _COMPILED


def _host_inputs(x, mask, Wqkv, Wout):
    slopes = _alibi_slopes(H)
    xf = np.ascontiguousarray(np.asarray(x, np.float32).reshape(B * S, HID))
    xT = np.ascontiguousarray(xf.T).astype(bf16).reshape(8, 128, 4096)
    maskf = np.asarray(mask, np.float32).reshape(B * S)
    mk = np.ascontiguousarray(maskf.reshape(32, 128).T)  # [128p, 32c]

    kk = np.arange(128, dtype=np.float32)[:, None]
    qq = np.arange(512, dtype=np.float32)[None, :]
    base = qq - kk  # [128, 512]

    Wqkv = np.asarray(Wqkv, np.float32)
    Wout = np.asarray(Wout, np.float32)

    in_maps = []
    for c in range(NCORES):
        h0 = 2 * c
        wq = np.ascontiguousarray(Wqkv[:, h0 * 64: h0 * 64 + 128]
                                  ).astype(bf16).reshape(8, 128, 128)
        wk = np.ascontiguousarray(Wqkv[:, 1024 + h0 * 64: 1024 + h0 * 64 + 128]
                                  ).astype(bf16).reshape(8, 128, 128)
        wv = np.ascontiguousarray(Wqkv[:, 2048 + h0 * 64: 2048 + h0 * 64 + 128]
                                  ).astype(bf16).reshape(8, 128, 128)
        wo = np.ascontiguousarray(Wout[c * 128:(c + 1) * 128, :]).astype(bf16)
        eb = np.empty((128, NEB * 1024), np.float32)
        for hl in range(2):
            sl = float(slopes[h0 + hl])
            for idx in range(NEB):
                if idx < 4:
                    bias = -sl * np.maximum(base - 128.0 * idx, 0.0)
                else:
                    bias = -sl * (base + 128.0 * (idx - 3))
                eb[:, idx * 1024 + hl * 512: idx * 1024 + (hl + 1) * 512] = np.exp(bias)
        in_maps.append({
            "xT": xT, "wq": wq, "wk": wk, "wv": wv, "wout": wo,
            "ebias": eb.astype(bf16), "maskf": mk,
        })
    return in_maps


def kernel(x, mask, Wqkv, Wout, trace=False):
    from concourse.bass_utils import run_bass_kernel_spmd

    nc = _get_nc()
    in_maps = _host_inputs(x, mask, Wqkv, Wout)
    res = run_bass_kernel_spmd(nc, in_maps, core_ids=list(range(NCORES)), trace=trace)
    results = res.results

    out = np.zeros((B * S, HID), np.float32)
    k_cache = np.empty((B * S, HID), np.float32)
    v_cache = np.empty((B * S, HID), np.float32)
    for c in range(NCORES):
        r = results[c]
        out += r["out_p"].astype(np.float32).reshape(B * S, HID)
        ko = r["k_out"].astype(np.float32)  # [64, 8192]
        for hl in range(2):
            k_cache[:, c * 128 + hl * 64: c * 128 + (hl + 1) * 64] = \
                ko[:, hl * 4096:(hl + 1) * 4096].T
        v_cache[:, c * 128:(c + 1) * 128] = \
            r["v_out"].astype(np.float32).transpose(1, 0, 2).reshape(B * S, 128)
    out = out.reshape(B, S, HID)
    k_cache = k_cache.reshape(B, S, HID)
    v_cache = v_cache.reshape(B, S, HID)
    if trace:
        return (out, k_cache, v_cache), res
    return out, k_cache, v_cache


# revision 16
# speedup vs baseline: 1.6435x; 1.2088x over previous
"""ALiBi attention (B=2, S=2048, H=16, Dh=64) on 8 TRN2 NeuronCores.

Sharding: head-parallel attention (2 heads x 2 batches per core), qkv
column-sharded, out-projection K-sharded (per-core partial summed on host).
No collectives. All heavy matmuls in bf16 with f32 PSUM accumulation.

One SPMD graph shared by all 8 cores: everything slope/head-dependent
(exp(ALiBi-bias) tiles) arrives via per-core inputs.

Softmax is computed in transposed score layout [k,q]; the denominator
falls out of the PV matmul via a mask column appended to V. The ALiBi
bias is applied multiplicatively AFTER exp (probs = exp(s)*exp(bias))
on the otherwise-idle GPSIMD engine; exp(bias) tiles are host-precomputed
per (head, tile-diagonal-offset) — multiplicative bf16 error is ~0.4%
with no cancellation hazard.
"""

import math
import numpy as np
import ml_dtypes

bf16 = ml_dtypes.bfloat16

HID, H, DH = 1024, 16, 64
B, S = 2, 2048
NCORES = 8
NEB = 16  # exp-bias tile-pair classes: 4 mixed (delta 0..-384) + 12 linear (128..1536)


def _alibi_slopes(n_head):
    main = 2 ** int(math.log2(n_head))
    m = (2.0 ** (-8.0 / main)) ** np.arange(1, 1 + main)
    if main < n_head:
        intra = (2.0 ** (-4.0 / main)) ** np.arange(1, 1 + 2 * (n_head - main), 2)
        m = np.concatenate([m, intra])
    return m.astype(np.float32)


def _eb_idx(delta):
    """exp-bias class index for tile diagonal offset delta, or None if bias==0."""
    if delta <= -512:
        return None
    if delta >= 128:
        return delta // 128 + 3  # 4..15
    return (-delta) // 128  # 0..3


def build_nc():
    import concourse.tile as tile
    from concourse import bacc, mybir

    f32 = mybir.dt.float32
    bf = mybir.dt.bfloat16
    AF = mybir.ActivationFunctionType

    nc = bacc.Bacc("TRN2", target_bir_lowering=False, debug=False,
                   enable_asserts=False, num_devices=NCORES)

    # ---- DRAM I/O ----
    xT_d = nc.dram_tensor("xT", [8, 128, 4096], bf, kind="ExternalInput").ap()
    wq_d = nc.dram_tensor("wq", [8, 128, 128], bf, kind="ExternalInput").ap()
    wk_d = nc.dram_tensor("wk", [8, 128, 128], bf, kind="ExternalInput").ap()
    wv_d = nc.dram_tensor("wv", [8, 128, 128], bf, kind="ExternalInput").ap()
    wout_d = nc.dram_tensor("wout", [128, 1024], bf, kind="ExternalInput").ap()
    eb_d = nc.dram_tensor("ebias", [128, NEB * 1024], bf, kind="ExternalInput").ap()
    mk_d = nc.dram_tensor("maskf", [128, 32], f32, kind="ExternalInput").ap()

    out_d = nc.dram_tensor("out_p", [2, 16, 128, 1024], bf, kind="ExternalOutput").ap()
    k_d = nc.dram_tensor("k_out", [128, 4096], bf, kind="ExternalOutput").ap()
    v_d = nc.dram_tensor("v_out", [128, 32, 128], bf, kind="ExternalOutput").ap()

    with tile.TileContext(nc) as tc:
        import contextlib
        with contextlib.ExitStack() as ctx:
            persist = ctx.enter_context(tc.tile_pool(name="persist", bufs=1))
            qT = persist.tile([128, 4096], bf, tag="qT")
            kT = persist.tile([128, 4096], bf, tag="kT")
            v_sb = persist.tile([128, 32 * 130], bf, tag="v_sb")
            attn_T = persist.tile([128, 4096], bf, tag="attn_T")
            wout_sb = persist.tile([128, 1024], bf, tag="wout")
            eb_sb = persist.tile([128, NEB * 1024], bf, tag="eb")
            maskf = persist.tile([128, 32], f32, tag="maskf")
            atst = persist.tile([64, 8 * 512], f32, tag="atst")   # numerators (1 batch)
            # denominators: slot (qc,hl) -> partition 32*qc, col hl*512
            # (compute writes must start at 32-aligned partitions)
            den = persist.tile([128, 1024], f32, tag="den")
            rcp = persist.tile([128, 1024], f32, tag="rcp")
            ones_sb = persist.tile([1, 64], f32, tag="ones")

            xw = ctx.enter_context(tc.tile_pool(name="xw", bufs=1))
            xT_sb = xw.tile([128, 8 * 4096], bf, tag="xT")
            wq_sb = xw.tile([128, 8 * 128], bf, tag="wq")
            wk_sb = xw.tile([128, 8 * 128], bf, tag="wk")
            wv_sb = xw.tile([128, 8 * 128], bf, tag="wv")

            work = ctx.enter_context(tc.tile_pool(name="work", bufs=3))
            work2 = ctx.enter_context(tc.tile_pool(name="work2", bufs=2))
            ps_sc = ctx.enter_context(tc.tile_pool(name="ps_sc", bufs=2, space="PSUM"))
            ps_ot = ctx.enter_context(tc.tile_pool(name="ps_ot", bufs=2, space="PSUM"))
            ps_ms = ctx.enter_context(tc.tile_pool(name="ps_ms", bufs=2, space="PSUM"))

            # ---- input DMAs (weights first, then x for batch 0, then rest) ----
            for h in range(8):
                nc.sync.dma_start(wq_sb[:, h * 128:(h + 1) * 128], wq_d[h])
                nc.sync.dma_start(wk_sb[:, h * 128:(h + 1) * 128], wk_d[h])
                nc.sync.dma_start(wv_sb[:, h * 128:(h + 1) * 128], wv_d[h])
            for h in range(8):
                nc.sync.dma_start(xT_sb[:, h * 4096: h * 4096 + 2048], xT_d[h, :, 0:2048])
            nc.sync.dma_start(wout_sb[:], wout_d[:])
            nc.sync.dma_start(maskf[:], mk_d[:])
            for h in range(8):
                nc.sync.dma_start(xT_sb[:, h * 4096 + 2048: (h + 1) * 4096],
                                  xT_d[h, :, 2048:4096])
            for j in range(4):
                nc.sync.dma_start(eb_sb[:, j * 4096:(j + 1) * 4096],
                                  eb_d[:, j * 4096:(j + 1) * 4096])

            # mask columns of v_sb (positions 64 and 129 of each 130-chunk)
            v3 = v_sb.rearrange("p (c w) -> p c w", w=130)
            mk3 = maskf.rearrange("p (c o) -> p c o", o=1)
            nc.vector.tensor_copy(v3[:, :, 64:65], mk3[:])
            nc.vector.tensor_copy(v3[:, :, 129:130], mk3[:])
            nc.vector.memset(den[:], 1.0)  # unused slots stay finite for reciprocal
            nc.vector.memset(ones_sb[:], 1.0)

            def qkv_phase(b):
                for w_sb, dst in ((wq_sb, qT), (wk_sb, kT)):
                    for n in range(4 * b, 4 * b + 4):  # 512-row chunks
                        ps = ps_ms.tile([128, 512], f32, tag="ms", name=f"qk{b}_{n}")
                        for h in range(8):
                            nc.tensor.matmul(
                                ps[:],
                                lhsT=w_sb[:, h * 128:(h + 1) * 128],
                                rhs=xT_sb[:, h * 4096 + n * 512: h * 4096 + (n + 1) * 512],
                                start=(h == 0), stop=(h == 7))
                        nc.vector.tensor_copy(
                            dst[:, n * 512:(n + 1) * 512], ps[:])
                vo = None
                for r in range(16 * b, 16 * b + 16):  # 128-row chunks
                    if r % 4 == 0:
                        vo = work2.tile([128, 512], bf, tag="vo", name=f"vo{r}")
                    ps = ps_ms.tile([128, 128], f32, tag="ms", name=f"v{b}_{r}")
                    for h in range(8):
                        nc.tensor.matmul(
                            ps[:],
                            lhsT=xT_sb[:, h * 4096 + r * 128: h * 4096 + r * 128 + 128],
                            rhs=wv_sb[:, h * 128:(h + 1) * 128],
                            start=(h == 0), stop=(h == 7))
                    for hl in range(2):
                        nc.vector.tensor_scalar_mul(
                            v3[:, r, hl * 65: hl * 65 + 64],
                            ps[:, hl * 64:(hl + 1) * 64],
                            maskf[:, r:r + 1])
                    nc.any.tensor_copy(vo[:, (r % 4) * 128:(r % 4) * 128 + 128], ps[:])
                    if r % 4 == 3:
                        nc.sync.dma_start(v_d[:, r - 3:r + 1, :],
                                          vo.rearrange("p (c w) -> p c w", w=128)[:])

            def attention_phase(b):
                for qc in range(4):
                    q0 = b * 2048 + qc * 512
                    otiles = [ps_ot.tile([65, 512], f32, tag="ot", name=f"ot{b}_{qc}_{hl}")
                              for hl in range(2)]
                    for kc in range(16):
                        delta = qc * 512 - kc * 128
                        ei = _eb_idx(delta)
                        sw = ps_sc.tile([128, 1024], f32, tag="sc", name=f"sw{b}_{qc}_{kc}")
                        koff = b * 2048 + kc * 128
                        qoff = b * 2048 + qc * 512
                        for hl in range(2):
                            nc.tensor.matmul(
                                sw[:, hl * 512:(hl + 1) * 512],
                                lhsT=kT[hl * 64:(hl + 1) * 64, koff:koff + 128],
                                rhs=qT[hl * 64:(hl + 1) * 64, qoff:qoff + 512],
                                start=True, stop=True)
                        pw = work.tile([128, 1024], bf, tag="pw", name=f"pw{b}_{qc}_{kc}")
                        nc.scalar.activation(pw[:], sw[:], AF.Exp)
                        if ei is not None:
                            pb = work.tile([128, 1024], bf, tag="pb", name=f"pb{b}_{qc}_{kc}")
                            eng = nc.gpsimd if kc % 2 == 0 else nc.vector
                            eng.tensor_mul(pb[:], pw[:],
                                           eb_sb[:, ei * 1024:(ei + 1) * 1024])
                            src = pb
                        else:
                            src = pw
                        ch = b * 16 + kc
                        for hl in range(2):
                            nc.tensor.matmul(
                                otiles[hl][:],
                                lhsT=v_sb[:, ch * 130 + hl * 65: ch * 130 + hl * 65 + 65],
                                rhs=src[:, hl * 512:(hl + 1) * 512],
                                start=(kc == 0), stop=(kc == 15))
                    for hl in range(2):
                        slot = qc * 2 + hl
                        nc.vector.tensor_copy(
                            den[32 * qc:32 * qc + 1, hl * 512:(hl + 1) * 512],
                            otiles[hl][64:65, :])
                        nc.vector.tensor_copy(atst[:, slot * 512:(slot + 1) * 512],
                                              otiles[hl][0:64, :])

            def normalize_and_outproj(b):
                nc.vector.reciprocal(rcp[:], den[:])
                for qc in range(4):
                    q0 = b * 2048 + qc * 512
                    for hl in range(2):
                        slot = qc * 2 + hl
                        rc0 = work.tile([1, 512], f32, tag="rc0", name=f"rc{b}_{slot}")
                        nc.vector.tensor_copy(
                            rc0[:], rcp[32 * qc:32 * qc + 1, hl * 512:(hl + 1) * 512])
                        bc = ps_ms.tile([64, 512], f32, tag="ms", name=f"bc{b}_{slot}")
                        nc.tensor.matmul(bc[:], lhsT=ones_sb[:], rhs=rc0[:],
                                         start=True, stop=True)
                        nc.vector.tensor_mul(
                            attn_T[hl * 64:(hl + 1) * 64, q0:q0 + 512],
                            atst[:, slot * 512:(slot + 1) * 512], bc[:])
                for r in range(16):
                    st = work.tile([128, 1024], bf, tag="ostage", name=f"st{b}_{r}")
                    for n2 in range(2):
                        ps = ps_ms.tile([128, 512], f32, tag="ms", name=f"op{b}_{r}_{n2}")
                        nc.tensor.matmul(
                            ps[:],
                            lhsT=attn_T[:, b * 2048 + r * 128: b * 2048 + r * 128 + 128],
                            rhs=wout_sb[:, n2 * 512:(n2 + 1) * 512],
                            start=True, stop=True)
                        nc.any.tensor_copy(st[:, n2 * 512:(n2 + 1) * 512], ps[:])
                    nc.sync.dma_start(out_d[b, r], st[:])

            qkv_phase(0)
            attention_phase(0)
            qkv_phase(1)
            nc.sync.dma_start(k_d[:], kT[:])
            normalize_and_outproj(0)
            attention_phase(1)
            normalize_and_outproj(1)

    nc.compile()
    return nc


_COMPILED = None


def _get_nc():
    global _COMPILED
    if _COMPILED is None:
        _COMPILED = build_nc()
    return _COMPILED


def _host_inputs(x, mask, Wqkv, Wout):
    slopes = _alibi_slopes(H)
    xf = np.ascontiguousarray(np.asarray(x, np.float32).reshape(B * S, HID))
    xT = np.ascontiguousarray(xf.T).astype(bf16).reshape(8, 128, 4096)
    maskf = np.asarray(mask, np.float32).reshape(B * S)
    mk = np.ascontiguousarray(maskf.reshape(32, 128).T)  # [128p, 32c]

    kk = np.arange(128, dtype=np.float32)[:, None]
    qq = np.arange(512, dtype=np.float32)[None, :]
    base = qq - kk  # [128, 512]

    Wqkv = np.asarray(Wqkv, np.float32)
    Wout = np.asarray(Wout, np.float32)

    in_maps = []
    for c in range(NCORES):
        h0 = 2 * c
        wq = np.ascontiguousarray(Wqkv[:, h0 * 64: h0 * 64 + 128]
                                  ).astype(bf16).reshape(8, 128, 128)
        wk = np.ascontiguousarray(Wqkv[:, 1024 + h0 * 64: 1024 + h0 * 64 + 128]
                                  ).astype(bf16).reshape(8, 128, 128)
        wv = np.ascontiguousarray(Wqkv[:, 2048 + h0 * 64: 2048 + h0 * 64 + 128]
                                  ).astype(bf16).reshape(8, 128, 128)
        wo = np.ascontiguousarray(Wout[c * 128:(c + 1) * 128, :]).astype(bf16)
        eb = np.empty((128, NEB * 1024), np.float32)
        for hl in range(2):
            sl = float(slopes[h0 + hl])
            for idx in range(NEB):
                if idx < 4:
                    bias = -sl * np.maximum(base - 128.0 * idx, 0.0)
                else:
                    bias = -sl * (base + 128.0 * (idx - 3))
                eb[:, idx * 1024 + hl * 512: idx * 1024 + (hl + 1) * 512] = np.exp(bias)
        in_maps.append({
            "xT": xT, "wq": wq, "wk": wk, "wv": wv, "wout": wo,
            "ebias": eb.astype(bf16), "maskf": mk,
        })
    return in_maps


def kernel(x, mask, Wqkv, Wout, trace=False):
    from concourse.bass_utils import run_bass_kernel_spmd

    nc = _get_nc()
    in_maps = _host_inputs(x, mask, Wqkv, Wout)
    res = run_bass_kernel_spmd(nc, in_maps, core_ids=list(range(NCORES)), trace=trace)
    results = res.results

    out = np.zeros((B * S, HID), np.float32)
    k_cache = np.empty((B * S, HID), np.float32)
    v_cache = np.empty((B * S, HID), np.float32)
    for c in range(NCORES):
        r = results[c]
        out += r["out_p"].astype(np.float32).reshape(B * S, HID)
        k_cache[:, c * 128:(c + 1) * 128] = r["k_out"].astype(np.float32).T
        v_cache[:, c * 128:(c + 1) * 128] = \
            r["v_out"].astype(np.float32).transpose(1, 0, 2).reshape(B * S, 128)
    out = out.reshape(B, S, HID)
    k_cache = k_cache.reshape(B, S, HID)
    v_cache = v_cache.reshape(B, S, HID)
    if trace:
        return (out, k_cache, v_cache), res
    return out, k_cache, v_cache
